# revision 29
# baseline (speedup 1.0000x reference)
"""Trainium2 Bass kernel for the PixelCNN-style decoder (nn_Decoder_14431090114949).

Data parallel over 8 NeuronCores: the fused (b*t)=8 batch axis is split one
sample per core; params are replicated.

Per-core algorithm (all resident in SBUF):
  x: (32, 128, 128), frame: (1, 128, 128) -> logits (256, 128, 128)
  4 RMBs: 1x1 conv in -> 2 masked-conv MUs -> 1x1 conv out (+skip).

Layout: channels on partitions, pixels on the free dim in a padded 130x130
flat image (1-px border). Masked 3x3 conv = sum of shifted 1x1 matmuls; two
taps are packed per matmul (K=128 = 64ch x 2 taps) via a +130-row-shifted
copy of h kept in partitions 64:128 of the conv buffer. M=128 packs two of
the four MU convs per PSUM group: [g2;g1] and [g3;u'].

All transcendentals are sigmoids (tanh(x) = 2 sig(2x) - 1, scale factors
folded into host-packed weights); consecutive super-tiles' s = g2*h+g3*u
land on opposite partition halves so one 128-lane sigmoid covers a pair.
MU outputs are stored at half scale ((t'-.5)*g1); the next stage's weights
are pre-doubled to compensate.
"""

import sys

sys.path.insert(0, "/opt/trn_rl_repo")

import numpy as np

import concourse.bass as bass  # noqa: F401
import concourse.mybir as mybir
from concourse import bacc, bass_utils
from concourse.tile import TileContext

H = W = 128
PW = 130
PH = 130
L = PW * PH  # 16900
NPIX = H * W  # 16384
RST = 8          # rows per super-tile
FD = RST * W     # 1024
NST = H // RST   # 16 super-tiles
PFD = 2 * FD     # pair free dim (2048)
MMF = 512        # matmul free dim (one PSUM bank of fp32)
MMR = MMF // W   # rows per matmul (4)

C_IN = 32
C_OUT = 256
N_RMB = 4

FP16 = mybir.dt.float16
F32 = mybir.dt.float32
AF = mybir.ActivationFunctionType
ALU = mybir.AluOpType

# Mask-A taps: (-1,-1),(-1,0),(-1,1),(0,-1); mask B adds (0,0).
# Tap-pair passes (base flat delta, [(k_block_start, tap)]); block1 works
# because hbuf[64+c, q] = h[c, q+130].
PASSES_A = [(-131, [(0, (-1, -1)), (64, (0, -1))]), (-130, [(0, (-1, 0))]),
            (-129, [(0, (-1, 1))])]
PASSES_B = [(-131, [(0, (-1, -1)), (64, (0, -1))]),
            (-130, [(0, (-1, 0)), (64, (0, 0))]), (-129, [(0, (-1, 1))])]
PASS_K = {"A": [128, 64, 64], "B": [128, 128, 64]}
PASS_D = [-131, -130, -129]


def _mu_group_lhst(wa, wb, passes):
    """lhsT blocks [(K, 128)] packing convs (a -> psum 0:64, b -> 64:128).

    hbuf rows 0:64 hold h at FULL scale, rows 64:128 hold h/2 -> the upper
    K block's weights are doubled.
    """
    out = []
    for _base, blocks in passes:
        K = max(b for b, _ in blocks) + 64
        lhsT = np.zeros((K, 128), np.float32)
        for bstart, (dy, dx) in blocks:
            ks = 2.0 if bstart == 64 else 1.0
            lhsT[bstart:bstart + 64, 0:64] = ks * wa[:, :, dy + 1, dx + 1].T
            lhsT[bstart:bstart + 64, 64:128] = ks * wb[:, :, dy + 1, dx + 1].T
        out.append(lhsT)
    return out


def pack_params(params):
    """Pack weights into [128, WCOLS] f32 (cast to bf16 later) + biases."""
    wcols = []
    bias_cols = []
    wmeta = {}
    bmeta = {}

    def add_w(name, lhsT):
        K, M = lhsT.shape
        col = sum(c.shape[1] for c in wcols)
        pad = np.zeros((128, M), np.float32)
        pad[:K] = lhsT
        wcols.append(pad)
        wmeta[name] = (col, K, M)

    def add_b(name, vec, row0=0):
        col = len(bias_cols)
        pad = np.zeros((128,), np.float32)
        pad[row0:row0 + len(vec)] = vec
        bias_cols.append(pad)
        bmeta[name] = col

    # x in SBUF accumulates only conv outputs; each skipped b_out is folded
    # into the NEXT rmb's input bias: b_in'_{i} = b_in_i + W_in_i @ sum_{j<i}
    # b_out_j.  hbuf stores h at HALF scale: every weight reading hbuf is
    # pre-doubled, stage_in drains with scale=0.5.
    cum_bout = np.zeros((C_IN,), np.float32)
    for i, p in enumerate(params):
        passes = PASSES_A if i == 0 else PASSES_B
        w_in = np.asarray(p["w_in"])[:, :, 0, 0]  # (64, in_ch)
        b_in = np.asarray(p["b_in"]) + w_in[:, :C_IN] @ cum_bout
        add_w(f"r{i}_in", w_in.T.astype(np.float32))
        add_b(f"r{i}_bin", b_in)
        for m in (0, 1):
            mu = p[f"mu{m}"]
            # group1 = [g1; g2] (g2 on the upper half so m1 can pair with the
            # half-scale upper hbuf copy); u' = sig(2*(c4+b4)) -> w4,b4 x2.
            w1 = np.asarray(mu["w1"])
            w2 = np.asarray(mu["w2"])
            w3 = np.asarray(mu["w3"])
            w4 = 2.0 * np.asarray(mu["w4"])
            b4 = 2.0 * np.asarray(mu["b4"])
            for j, lhsT in enumerate(_mu_group_lhst(w1, w2, passes)):
                add_w(f"r{i}m{m}_g21_p{j}", lhsT)
            for j, lhsT in enumerate(_mu_group_lhst(w3, w4, passes)):
                add_w(f"r{i}m{m}_g3u_p{j}", lhsT)
            add_b(f"r{i}m{m}_b21",
                  np.concatenate([np.asarray(mu["b1"]), np.asarray(mu["b2"])]))
            add_b(f"r{i}m{m}_b3u",
                  np.concatenate([np.asarray(mu["b3"]), b4]))
        # w_out reads hbuf's full-scale lower half
        w_out = np.asarray(p["w_out"])[:, :, 0, 0]  # (out_ch, 64)
        b_out = np.asarray(p["b_out"])
        if i < N_RMB - 1:
            if i == N_RMB - 2:
                # x_3 is only read by rmb3's stage_in -> never materialize it;
                # rmb3's stage_in adds (W_in3 @ w_out2) @ h'_2 instead.
                w_in3 = np.asarray(params[i + 1]["w_in"])[:, :, 0, 0]
                add_w("r3_inV", (w_in3 @ w_out).T.astype(np.float32))
            else:
                add_w(f"r{i}_out", w_out.T.astype(np.float32))
            cum_bout = cum_bout + b_out  # folded into next rmb's b_in
        else:
            add_w(f"r{i}_out_lo", w_out[:128].T.astype(np.float32))
            add_w(f"r{i}_out_hi", w_out[128:].T.astype(np.float32))
            add_b(f"r{i}_bout_lo", b_out[:128])
            add_b(f"r{i}_bout_hi", b_out[128:])

    wts = np.concatenate(wcols, axis=1).astype(np.float32)
    biases = np.stack(bias_cols, axis=1).astype(np.float32)
    return wts, biases, wmeta, bmeta


def build_kernel(wmeta, bmeta, wcols_total, bcols_total):
    nc = bacc.Bacc("TRN2", target_bir_lowering=False, debug=False)

    xf_d = nc.declare_dram_parameter("xf", [33, NPIX], FP16, isOutput=False)
    wts_d = nc.declare_dram_parameter("wts", [128, wcols_total], FP16,
                                      isOutput=False)
    bias_d = nc.declare_dram_parameter("biases", [128, bcols_total], F32,
                                       isOutput=False)
    out_d = nc.declare_dram_parameter("out", [C_OUT, NPIX], F32, isOutput=True)

    with TileContext(nc) as tc:
        with (
            tc.tile_pool(name="persist", bufs=1) as persist,
            tc.tile_pool(name="work", bufs=3) as work,
            tc.tile_pool(name="outp", bufs=2) as outp,
            tc.tile_pool(name="psA", bufs=2, space="PSUM") as psA,
            tc.tile_pool(name="psB", bufs=1, space="PSUM") as psB,
            tc.tile_pool(name="psO", bufs=1, space="PSUM") as psO,
        ):
            hbuf_a = persist.tile([128, L], FP16)
            hbuf_b = persist.tile([128, L], FP16)
            xf = persist.tile([64, NPIX], FP16)
            wts = persist.tile([128, wcols_total], FP16)
            biases = persist.tile([128, bcols_total], F32)

            warm = persist.tile([1, 16], F32)
            nc.scalar.activation(warm[:], warm[:], AF.Sigmoid)
            for hb in (hbuf_a, hbuf_b):
                hv = hb.rearrange("p (r c) -> p r c", c=PW)
                # top + bottom pad rows, and left+right pad columns of every
                # row; interior cells are always written before being read.
                nc.gpsimd.memset(hv[:, 0, :], 0.0)
                nc.gpsimd.memset(hv[:, PH - 1, :], 0.0)
                nc.gpsimd.memset(hv[:, :, 0], 0.0)
                nc.gpsimd.memset(hv[:, :, PW - 1], 0.0)
            nc.sync.dma_start(out=xf[0:33], in_=xf_d[:])
            nc.sync.dma_start(out=wts[:], in_=wts_d[:])
            nc.sync.dma_start(out=biases[:], in_=bias_d[:])

            def w_ap(name):
                col, K, M = wmeta[name]
                return wts[0:K, col:col + M]

            def b_ap(name, p0=0, p1=None):
                col = bmeta[name]
                if p1 is None:
                    p1 = p0 + 64
                return biases[p0:p1, col:col + 1]

            hview = {
                id(hbuf_a): hbuf_a.rearrange("p (r c) -> p r c", c=PW),
                id(hbuf_b): hbuf_b.rearrange("p (r c) -> p r c", c=PW),
            }

            def win(buf, st, delta, kp0, kp1, rows=RST, row0=0):
                """Strided window [kp0:kp1, rows, 128] at super-tile st."""
                q0 = ((st * RST + row0) + 1) * PW + 1 + delta
                a, b = divmod(q0, PW)
                return hview[id(buf)][kp0:kp1, a:a + rows, b:b + 128]

            def r3(ap, c=W):
                return ap.rearrange("p (r c) -> p r c", c=c)

            # -------- stages (all emitted per super-tile pair) --------

            def stage_in(rmb, dst, h2src=None):
                """h/2 = 0.5*(W_in @ xin + b_in') -> dst conv buffer.

                h2src: for rmb3, the buffer holding h'_2; its contribution
                (W_in3 @ w_out2) @ h'_2 replaces the never-materialized x_3.
                """
                K = 33 if rmb == 0 else 32
                wap = w_ap(f"r{rmb}_in")
                vap = w_ap("r3_inV") if h2src is not None else None
                for st in range(NST):
                    p0 = st * FD
                    pin = psA.tile([64, FD], F32, tag="psA", name="pin")
                    for hf in range(FD // MMF):
                        nc.tensor.matmul(
                            pin[:, hf * MMF:(hf + 1) * MMF],
                            wap,
                            xf[0:K, p0 + hf * MMF:p0 + (hf + 1) * MMF],
                            start=True, stop=(vap is None))
                        if vap is not None:
                            nc.tensor.matmul(
                                pin[:, hf * MMF:(hf + 1) * MMF],
                                vap,
                                win(h2src, st, 0, 0, 64, rows=MMR,
                                    row0=hf * MMR),
                                start=False, stop=True)
                    # alternate the drain engine to balance ACT vs DVE load
                    if rmb % 2 == 0:
                        nc.scalar.activation(
                            win(dst, st, 0, 0, 64), r3(pin),
                            AF.Identity, bias=b_ap(f"r{rmb}_bin"))
                    else:
                        nc.vector.tensor_scalar(
                            win(dst, st, 0, 0, 64), r3(pin),
                            1.0, b_ap(f"r{rmb}_bin"), ALU.mult, ALU.add)
                    nc.gpsimd.tensor_scalar(
                        win(dst, st, -130, 64, 128),
                        win(dst, st, 0, 0, 64), 0.5, None, ALU.mult)

            def stage_mu(rmb, m, src, dst, only_pair=None):
                mask = "A" if rmb == 0 else "B"
                ks = PASS_K[mask]
                wl21 = [w_ap(f"r{rmb}m{m}_g21_p{j}") for j in range(3)]
                wl3u = [w_ap(f"r{rmb}m{m}_g3u_p{j}") for j in range(3)]
                prs = range(NST // 2) if only_pair is None else [only_pair]
                for pr in prs:
                    st0 = 2 * pr
                    g21 = work.tile([128, PFD], FP16, tag="g21", name="g21",
                                    bufs=4)
                    g3u = work.tile([128, PFD], FP16, tag="g3u", name="g3u",
                                    bufs=4)
                    m1 = work.tile([64, PFD], FP16, tag="m1", name="m1",
                                   bufs=4)
                    m2 = work.tile([64, PFD], FP16, tag="m2", name="m2",
                                   bufs=4)
                    S = work.tile([128, FD], FP16, tag="s", name="s",
                                  bufs=4)
                    tp = work.tile([128, FD], FP16, tag="tp", name="tp",
                                   bufs=4)
                    # per-super-tile psum tiles (2 banks each, double-buffered)
                    for half, st in enumerate((st0, st0 + 1)):
                        hs = slice(half * FD, (half + 1) * FD)
                        p21 = psA.tile([128, FD], F32, tag="psA", name="p21")
                        p3u = psB.tile([128, FD], F32, tag="psB", name="p3u")
                        for ps, wl in ((p21, wl21), (p3u, wl3u)):
                            for hf in range(FD // MMF):
                                for j in range(3):
                                    nc.tensor.matmul(
                                        ps[:, hf * MMF:(hf + 1) * MMF],
                                        wl[j],
                                        win(src, st, PASS_D[j], 0, ks[j],
                                            rows=MMR, row0=hf * MMR),
                                        start=(j == 0), stop=(j == 2))
                        nc.scalar.activation(
                            g21[:, hs], p21[:], AF.Sigmoid,
                            bias=b_ap(f"r{rmb}m{m}_b21", 0, 128))
                        nc.scalar.activation(
                            g3u[:, hs], p3u[:], AF.Sigmoid,
                            bias=b_ap(f"r{rmb}m{m}_b3u", 0, 128))
                    # vp = u' - 0.5 = u/2 (into m2's slot; overwritten below)
                    nc.vector.tensor_scalar(
                        m2[:], g3u[64:128], 0.5, None, ALU.subtract)
                    # m1 = g2 * h/2 (g2 and the half-scale h copy share the
                    # upper partition half); m2 = g3 * u/2 -> co-scaled adds
                    nc.vector.tensor_mul(
                        r3(m1), r3(g21[64:128]),
                        win(src, st0, -130, 64, 128, rows=2 * RST))
                    nc.vector.tensor_mul(m2[:], g3u[0:64], m2[:])
                    # s/2; st0 -> S[64:128], st1 -> S[0:64]
                    nc.vector.tensor_add(S[64:128], m1[:, 0:FD], m2[:, 0:FD])
                    nc.vector.tensor_add(S[0:64], m1[:, FD:PFD],
                                         m2[:, FD:PFD])
                    # t = tanh(s) = tanh(2 * s/2) for the pair, 128 lanes
                    nc.scalar.activation(tp[:], S[:], AF.Tanh, scale=2.0)
                    # hop st0's t to the lower half (g1 lives at 0:64)
                    nc.vector.tensor_copy(g3u[0:64, 0:FD], tp[64:128])
                    # y = g1 * tanh(s) -> dst lower (full scale)
                    nc.vector.tensor_mul(
                        win(dst, st0, 0, 0, 64), r3(g21[0:64, 0:FD]),
                        r3(g3u[0:64, 0:FD]))
                    nc.vector.tensor_mul(
                        win(dst, st0 + 1, 0, 0, 64), r3(g21[0:64, FD:PFD]),
                        r3(tp[0:64]))
                    nc.gpsimd.tensor_scalar(
                        win(dst, st0, -130, 64, 128, rows=2 * RST),
                        win(dst, st0, 0, 0, 64, rows=2 * RST),
                        0.5, None, ALU.mult)

            def stage_out(rmb, src, only_sts=None):
                if rmb < N_RMB - 1:
                    wap = w_ap(f"r{rmb}_out")
                    for st in range(NST):
                        po = psO.tile([32, FD], F32, tag="psO", name="po")
                        for hf in range(FD // MMF):
                            nc.tensor.matmul(
                                po[:, hf * MMF:(hf + 1) * MMF],
                                wap,
                                win(src, st, 0, 0, 64, rows=MMR,
                                    row0=hf * MMR),
                                start=True, stop=True)
                        p0 = st * FD
                        # x += po  (b_out folded into the next rmb's b_in)
                        nc.vector.tensor_add(
                            xf[0:32, p0:p0 + FD], xf[0:32, p0:p0 + FD],
                            po[:])
                else:
                    for st in (range(NST) if only_sts is None else only_sts):
                        p0 = st * FD
                        for gi, gname in enumerate(("lo", "hi")):
                            po = psO.tile([128, FD], F32, tag="psO",
                                          name="po2")
                            for hf in range(FD // MMF):
                                nc.tensor.matmul(
                                    po[:, hf * MMF:(hf + 1) * MMF],
                                    w_ap(f"r{rmb}_out_{gname}"),
                                    win(src, st, 0, 0, 64, rows=MMR,
                                        row0=hf * MMR),
                                    start=True, stop=True)
                            ost = outp.tile([128, FD], F32, tag="ost",
                                            name="ost")
                            if gi == 0:
                                nc.scalar.activation(
                                    ost[:], po[:], AF.Identity,
                                    bias=b_ap(f"r{rmb}_bout_{gname}", 0, 128))
                            else:
                                nc.vector.tensor_scalar(
                                    ost[:], po[:], 1.0,
                                    b_ap(f"r{rmb}_bout_{gname}", 0, 128),
                                    ALU.mult, ALU.add)
                            nc.sync.dma_start(
                                out=out_d[gi * 128:(gi + 1) * 128,
                                          p0:p0 + FD],
                                in_=ost[:])

            cur, nxt = hbuf_a, hbuf_b
            for rmb in range(N_RMB):
                stage_in(rmb, cur, h2src=(nxt if rmb == N_RMB - 1 else None))
                stage_mu(rmb, 0, cur, nxt)
                if rmb == N_RMB - 1:
                    # interleave MU1 with the output stage so the ACT-bound
                    # tail overlaps MU1's compute (psO keeps slots separate)
                    for pr in range(NST // 2):
                        stage_mu(rmb, 1, nxt, cur, only_pair=pr)
                        stage_out(rmb, cur, only_sts=(2 * pr, 2 * pr + 1))
                else:
                    stage_mu(rmb, 1, nxt, cur)
                    if rmb != N_RMB - 2:
                        stage_out(rmb, cur)
                cur, nxt = nxt, cur

    nc.compile()
    return nc


_CACHE = {}


def kernel(inputs, targets, params):
    b, t, c, h, w = inputs.shape
    M = b * t
    X = np.asarray(inputs).reshape(M, c, h * w)
    F = np.asarray(targets).reshape(M, 1, h * w)

    wts, biases, wmeta, bmeta = pack_params(params)
    key = (wts.shape[1], biases.shape[1])
    if key not in _CACHE:
        _CACHE[key] = build_kernel(wmeta, bmeta, wts.shape[1], biases.shape[1])
    nc = _CACHE[key]

    wts_bf = wts.astype(np.float16)
    in_maps = []
    for n in range(M):
        xfa = np.concatenate([X[n], F[n]], axis=0).astype(np.float16)
        in_maps.append({"xf": xfa, "wts": wts_bf, "biases": biases})

    res = bass_utils.run_bass_kernel_spmd(nc, in_maps, list(range(M)))
    outs = [res.results[n]["out"].reshape(C_OUT, h, w) for n in range(M)]
    return np.stack(outs).reshape(b, t, C_OUT, h, w).astype(np.float32)
